# revision 1
# baseline (speedup 1.0000x reference)
"""Trainium2 Bass kernel for 8-head MultiHeadAttention (B=2, S=4096, E=512).

Sharding: 8 cores = 2 batches x 4 query-row chunks of 1024. Each core computes
all 8 heads for its (batch, q-range): QK^T scores are built TRANSPOSED
([k partitions, q free]) so the attention-value matmul needs no on-chip
transposes; the V projection is algebraically fused into the attention-value
matmul (ctx^T = Wv @ (Xv^T @ P^T)) so the raw `value` columns serve as the
stationary operand, with an appended ones-column producing the softmax
denominator for free. Softmax runs without max-subtraction (scores/8 are
bounded), the mask is applied multiplicatively after exp (bf16 on DVE).
"""
import sys
for _p in ('/root/.axon_site/_ro/trn_rl_repo', '/opt/trn_rl_repo'):
    if _p not in sys.path:
        sys.path.append(_p)

import numpy as np
import ml_dtypes

import concourse.bass as bass
import concourse.tile as tile
from concourse import bacc, mybir
from concourse import bass_utils

F32 = mybir.dt.float32
BF16 = mybir.dt.bfloat16
F32R = mybir.dt.float32r
AF = mybir.ActivationFunctionType
ALU = mybir.AluOpType

N_CORES = 8
B, S, E, H, DH = 2, 4096, 512, 8, 64
QLEN = S // 4          # 1024 q rows per core
KC = S // 128          # 32 k chunks
QW = QLEN // 512       # 2 q windows of 512

_CACHE = {}


def _build_module():
    nc = bacc.Bacc("TRN2", target_bir_lowering=False, debug=False,
                   enable_asserts=True, num_devices=N_CORES)

    xqT = nc.dram_tensor("xqT", [E, QLEN], BF16, kind="ExternalInput").ap()
    xkT = nc.dram_tensor("xkT", [E, S], BF16, kind="ExternalInput").ap()
    valp = nc.dram_tensor("valp", [S, H * 65], BF16, kind="ExternalInput").ap()
    maskT = nc.dram_tensor("maskT", [S, QLEN], BF16, kind="ExternalInput").ap()
    wqT = nc.dram_tensor("wqT", [DH, DH], BF16, kind="ExternalInput").ap()
    wkT = nc.dram_tensor("wkT", [DH, DH], BF16, kind="ExternalInput").ap()
    wvT = nc.dram_tensor("wvT", [DH, DH], BF16, kind="ExternalInput").ap()
    woT = nc.dram_tensor("woT", [E, E], BF16, kind="ExternalInput").ap()
    bo_b = nc.dram_tensor("bo_b", [128, E], F32, kind="ExternalInput").ap()
    out = nc.dram_tensor("out", [QLEN, E], F32, kind="ExternalOutput").ap()
    rscr = nc.dram_tensor("rscr", [32, 512], F32, kind="Internal").ap()

    with tile.TileContext(nc) as tc:
        _emit(tc, nc, xqT, xkT, valp, maskT, wqT, wkT, wvT, woT, bo_b, out, rscr)

    nc.compile()
    return nc


def _emit(tc, nc, xqT, xkT, valp, maskT, wqT, wkT, wvT, woT, bo_b, out, rscr):
    from contextlib import ExitStack
    ctx = ExitStack()
    const = ctx.enter_context(tc.tile_pool(name="const", bufs=1))
    kpool = ctx.enter_context(tc.tile_pool(name="kproj", bufs=1))
    qpool = ctx.enter_context(tc.tile_pool(name="qproj", bufs=4))
    xkst = ctx.enter_context(tc.tile_pool(name="xkst", bufs=2))
    ppool = ctx.enter_context(tc.tile_pool(name="p", bufs=6))
    usbp = ctx.enter_context(tc.tile_pool(name="usb", bufs=4))
    rcpool = ctx.enter_context(tc.tile_pool(name="rc", bufs=3))
    ospool = ctx.enter_context(tc.tile_pool(name="osb", bufs=2))
    psp = ctx.enter_context(tc.tile_pool(name="psp", bufs=2, space="PSUM"))
    uacc = ctx.enter_context(tc.tile_pool(name="uacc", bufs=2, space="PSUM"))
    utmp = ctx.enter_context(tc.tile_pool(name="utmp", bufs=2, space="PSUM"))

    # ---- resident mask tiles (loaded once) ----
    mask_res = [const.tile([128, QLEN], BF16, tag=f"mk{c}", name=f"mk{c}")
                for c in range(KC)]

    def load_masks():
        for c in range(KC):
            nc.sync.dma_start(mask_res[c], maskT[c * 128:(c + 1) * 128, :])

    # ---- constants (wq/wk immediately; heavy/late consts after proj0 loads) ----
    wq_sb = const.tile([DH, DH], BF16, tag="wq")
    nc.gpsimd.dma_start(wq_sb, wqT)
    wk_sb = const.tile([DH, DH], BF16, tag="wk")
    nc.gpsimd.dma_start(wk_sb, wkT)
    wv_sb = const.tile([DH, DH], BF16, tag="wv")
    wo_sb = []
    for pc in range(4):
        wo_sb.append(const.tile([128, E], BF16, tag=f"wo{pc}", name=f"wo{pc}"))
    bo_sb = const.tile([128, E], F32, tag="bo")
    ones_sb = const.tile([65, DH], BF16, tag="ones")
    nc.vector.memset(ones_sb, 1.0)

    def load_late_consts():
        nc.sync.dma_start(wv_sb, wvT)
        for pc in range(4):
            nc.sync.dma_start(wo_sb[pc], woT[pc * 128:(pc + 1) * 128, :])
        nc.sync.dma_start(bo_sb, bo_b)
    valp_t = []
    for c in range(KC):
        t = const.tile([128, H * 65], BF16, tag=f"vp{c}", name=f"vp{c}")
        nc.gpsimd.dma_start(t, valp[c * 128:(c + 1) * 128, :])
        valp_t.append(t)
    concatT = []
    for pair in range(4):
        concatT.append(const.tile([128, QLEN], BF16, tag=f"ct{pair}",
                                  name=f"ct{pair}"))

    kproj_sb = [None] * 4
    qproj_sb = [None] * 4

    xs = {}

    def proj_load(pair):
        xk0 = xkst.tile([DH, S], BF16, tag="xk", name=f"xk0_{pair}")
        nc.sync.dma_start(xk0, xkT[(2 * pair) * DH:(2 * pair + 1) * DH, :])
        xq0 = xkst.tile([DH, QLEN], BF16, tag="xq", name=f"xq0_{pair}")
        nc.sync.dma_start(xq0, xqT[(2 * pair) * DH:(2 * pair + 1) * DH, :])
        xk1 = xkst.tile([DH, S], BF16, tag="xk", name=f"xk1_{pair}")
        nc.sync.dma_start(xk1, xkT[(2 * pair + 1) * DH:(2 * pair + 2) * DH, :])
        xq1 = xkst.tile([DH, QLEN], BF16, tag="xq", name=f"xq1_{pair}")
        nc.sync.dma_start(xq1, xqT[(2 * pair + 1) * DH:(2 * pair + 2) * DH, :])
        xs[pair] = (xk0, xk1, xq0, xq1)
        kproj_sb[pair] = kpool.tile([128, S], BF16, tag=f"kp{pair}",
                                    name=f"kp{pair}")
        qproj_sb[pair] = qpool.tile([128, QLEN], BF16, tag="qp",
                                    name=f"qp{pair}")

    def proj_chunks(pair, fast_start=False):
        """Closures: 8 kproj chunks + 2 qproj chunks."""
        xk0, xk1, xq0, xq1 = xs[pair]
        kp_sb = kproj_sb[pair]
        qp_sb = qproj_sb[pair]

        def half(dst, srcs, w, h2, nm):
            def go():
                t = utmp.tile([128, 512], F32, tag="ut", name=nm)
                lo, hi = h2 * 64, (h2 + 1) * 64
                nc.tensor.matmul(t[lo:hi, :], lhsT=w, rhs=srcs[:, 0:512],
                                 start=True, stop=True)
                nc.vector.tensor_copy(dst[lo:hi, 0:512], t[lo:hi, :])
            return go

        def kchunk(kc):
            def go():
                kp = utmp.tile([128, 512], F32, tag="ut", name=f"kpp{pair}_{kc}")
                nc.tensor.matmul(kp[0:64, :], lhsT=wk_sb,
                                 rhs=xk0[:, kc * 512:(kc + 1) * 512],
                                 start=True, stop=True)
                nc.tensor.matmul(kp[64:128, :], lhsT=wk_sb,
                                 rhs=xk1[:, kc * 512:(kc + 1) * 512],
                                 start=True, stop=True)
                nc.vector.tensor_copy(kp_sb[:, kc * 512:(kc + 1) * 512], kp)
            return go

        def qchunk(qc):
            def go():
                qp = utmp.tile([128, 512], F32, tag="ut", name=f"qpp{pair}_{qc}")
                nc.tensor.matmul(qp[0:64, :], lhsT=wq_sb,
                                 rhs=xq0[:, qc * 512:(qc + 1) * 512],
                                 start=True, stop=True)
                nc.tensor.matmul(qp[64:128, :], lhsT=wq_sb,
                                 rhs=xq1[:, qc * 512:(qc + 1) * 512],
                                 start=True, stop=True)
                nc.vector.tensor_copy(qp_sb[:, qc * 512:(qc + 1) * 512], qp)
            return go

        if fast_start:
            return ([half(kp_sb, xk0, wk_sb, 0, "fk0"),
                     half(qp_sb, xq0, wq_sb, 0, "fq0"),
                     half(kp_sb, xk1, wk_sb, 1, "fk1"),
                     half(qp_sb, xq1, wq_sb, 1, "fq1")]
                    + [kchunk(kc) for kc in range(1, 8)]
                    + [qchunk(qc) for qc in range(1, QW)])
        return ([kchunk(0), qchunk(0)] + [kchunk(kc) for kc in range(1, 8)]
                + [qchunk(qc) for qc in range(1, QW)])

    def attn(pair, qw, trickle=(), last=False):
        trickle = list(trickle)
        kp_sb = kproj_sb[pair]
        qp_sb = qproj_sb[pair]
        U = [uacc.tile([65, 512], F32, tag="u", name=f"U{pair}_{qw}_{h2}")
             for h2 in range(2)]
        for kc in range(KC):
            ps = psp.tile([128, 1024], F32, tag="ps", name=f"ps{pair}_{qw}_{kc}")
            nc.tensor.matmul(
                ps[:, 0:512],
                lhsT=kp_sb[0:64, kc * 128:(kc + 1) * 128],
                rhs=qp_sb[0:64, qw * 512:(qw + 1) * 512],
                start=True, stop=True)
            nc.tensor.matmul(
                ps[:, 512:1024],
                lhsT=kp_sb[64:128, kc * 128:(kc + 1) * 128],
                rhs=qp_sb[64:128, qw * 512:(qw + 1) * 512],
                start=True, stop=True)
            pt = ppool.tile([128, 1024], BF16, tag="pt", name=f"pt{pair}_{qw}_{kc}")
            if kc == KC - 1:
                nc.scalar.activation(pt[:, 0:512], ps[:, 0:512],
                                     AF.Exp, bias=0.0, scale=0.125)
                nc.scalar.activation(pt[:, 512:1024], ps[:, 512:1024],
                                     AF.Exp, bias=0.0, scale=0.125)
            else:
                nc.scalar.activation(pt, ps, AF.Exp, bias=0.0, scale=0.125)
            # in-place mask multiply, mask broadcast across the head pair
            ms = mask_res[kc][:, qw * 512:(qw + 1) * 512]
            if kc == KC - 1:
                # split per-head so the U-closing matmuls start sooner
                for h2 in range(2):
                    h = 2 * pair + h2
                    nc.vector.tensor_mul(pt[:, h2 * 512:(h2 + 1) * 512],
                                         pt[:, h2 * 512:(h2 + 1) * 512], ms)
                    nc.tensor.matmul(
                        U[h2][:, :],
                        lhsT=valp_t[kc][:, h * 65:(h + 1) * 65],
                        rhs=pt[:, h2 * 512:(h2 + 1) * 512],
                        start=False, stop=True)
            else:
                mb = bass.AP(tensor=ms.tensor, offset=ms.offset,
                             ap=[ms.ap[0], [0, 2], [1, 512]])
                pv = pt.rearrange("p (h q) -> p h q", h=2)
                nc.vector.tensor_mul(pv, pv, mb)
                for h2 in range(2):
                    h = 2 * pair + h2
                    nc.tensor.matmul(
                        U[h2][:, :],
                        lhsT=valp_t[kc][:, h * 65:(h + 1) * 65],
                        rhs=pt[:, h2 * 512:(h2 + 1) * 512],
                        start=(kc == 0), stop=(kc == KC - 1))
            if trickle and kc % 2 == 1 and kc >= 11:
                trickle.pop(0)()
        for work in trickle:
            work()
        # tail: free the U PSUM banks immediately; everything downstream is
        # returned as closures and trickled into the next phase's k-loop so
        # the in-order PE stream never stalls on this dependency chain.
        usb = [usbp.tile([65, 512], F32, tag="usb", name=f"usb{pair}_{qw}_{h2}")
               for h2 in range(2)]
        for h2 in range(2):
            if last:
                nc.scalar.copy(usb[h2], U[h2])
            else:
                nc.vector.tensor_copy(usb[h2], U[h2])

        def norm_one(h2, qlo=0, qwidth=512):
            def go():
                rc = rcpool.tile([65, 512], BF16, tag="rc",
                                 name=f"rc{pair}_{qw}_{h2}_{qlo}")
                with nc.allow_low_precision(reason="1/D broadcast in bf16"):
                    nc.vector.reciprocal(rc[64:65, qlo:qlo + qwidth],
                                         usb[h2][64:65, qlo:qlo + qwidth])
                rb = utmp.tile([64, 512], F32, tag="ut",
                               name=f"rb{pair}_{qw}_{h2}_{qlo}")
                nc.tensor.matmul(rb[:, 0:qwidth], lhsT=ones_sb[64:65, :],
                                 rhs=rc[64:65, qlo:qlo + qwidth],
                                 start=True, stop=True)
                un = rcpool.tile([64, 512], BF16, tag="un",
                                 name=f"un{pair}_{qw}_{h2}_{qlo}")
                nc.vector.scalar_tensor_tensor(
                    un[:, 0:qwidth], usb[h2][0:64, qlo:qlo + qwidth], 1.0,
                    rb[:, 0:qwidth], ALU.mult, ALU.mult)
                cx = utmp.tile([128, 512], F32, tag="ut",
                               name=f"cx{pair}_{qw}_{h2}_{qlo}")
                nc.tensor.matmul(cx[h2 * 64:(h2 + 1) * 64, 0:qwidth],
                                 lhsT=wv_sb, rhs=un[:, 0:qwidth],
                                 start=True, stop=True)
                ceng = nc.scalar if last else nc.vector
                ceng.copy = getattr(ceng, 'copy', None)
                if last:
                    nc.scalar.copy(
                        concatT[pair][h2 * 64:(h2 + 1) * 64,
                                      qw * 512 + qlo:qw * 512 + qlo + qwidth],
                        cx[h2 * 64:(h2 + 1) * 64, 0:qwidth])
                else:
                    nc.vector.tensor_copy(
                        concatT[pair][h2 * 64:(h2 + 1) * 64,
                                      qw * 512 + qlo:qw * 512 + qlo + qwidth],
                        cx[h2 * 64:(h2 + 1) * 64, 0:qwidth])
            return go

        if last:
            return [norm_one(0, 0, 256), norm_one(1, 0, 256),
                    norm_one(0, 256, 256), norm_one(1, 256, 256)]
        return [norm_one(0), norm_one(1)]

    def outproj(qts):
        def one(qt):
            def go():
                op = utmp.tile([128, 512], F32, tag="ut", name=f"op{qt}")
                for pc in range(4):
                    nc.tensor.matmul(op,
                                     lhsT=concatT[pc][:, qt * 128:(qt + 1) * 128],
                                     rhs=wo_sb[pc],
                                     start=(pc == 0), stop=(pc == 3))
                osb = ospool.tile([128, E], F32, tag="osb", name=f"osb{qt}")
                nc.vector.scalar_tensor_tensor(osb, op, 1.0, bo_sb,
                                               ALU.mult, ALU.add)
                nc.sync.dma_start(out[qt * 128:(qt + 1) * 128, :], osb)
            return go
        return [one(qt) for qt in qts]

    # emission schedule: proj0 upfront; later projections and every phase
    # tail trickle into the ACT-bound attention k-loops
    proj_load(0)
    for work in proj_chunks(0):
        work()
    proj_load(1)
    load_masks()
    load_late_consts()
    t00 = attn(0, 0, trickle=proj_chunks(1))
    t01 = attn(0, 1, trickle=t00)
    proj_load(2)
    t10 = attn(1, 0, trickle=t01 + proj_chunks(2))
    t11 = attn(1, 1, trickle=t10)
    proj_load(3)
    t20 = attn(2, 0, trickle=t11 + proj_chunks(3))
    t21 = attn(2, 1, trickle=t20)
    t30 = attn(3, 0, trickle=t21)
    t31 = attn(3, 1, trickle=t30 + outproj(range(4)), last=True)
    qts = outproj(range(4, 8))
    t31[0]()
    t31[1]()
    qts[0]()
    qts[1]()
    t31[2]()
    t31[3]()
    qts[2]()
    qts[3]()

    ctx.close()


def _prep_inputs(key, query, value, mask, Wq, Wk, Wv, Wo, bo):
    bf16 = ml_dtypes.bfloat16
    key = np.asarray(key, np.float32)
    query = np.asarray(query, np.float32)
    value = np.asarray(value, np.float32)
    mask = np.asarray(mask)
    common = {
        "wqT": np.ascontiguousarray(np.asarray(Wq, np.float32).T).astype(bf16),
        "wkT": np.ascontiguousarray(np.asarray(Wk, np.float32).T).astype(bf16),
        "wvT": np.ascontiguousarray(np.asarray(Wv, np.float32).T).astype(bf16),
        "woT": np.ascontiguousarray(np.asarray(Wo, np.float32).T).astype(bf16),
        "bo_b": np.ascontiguousarray(
            np.broadcast_to(np.asarray(bo, np.float32), (128, E))),
    }
    maskT = np.ascontiguousarray(
        (mask[0, 0] != 0).astype(np.float32).T.astype(bf16))  # [k, q]
    per_b = {}
    for b in range(B):
        vp = np.ones((S, H, 65), np.float32)
        vp[:, :, :64] = value[b].reshape(S, H, DH)
        per_b[b] = {
            "xkT": np.ascontiguousarray(key[b].T).astype(bf16),
            "valp": np.ascontiguousarray(vp.reshape(S, H * 65).astype(bf16)),
            "qT": query[b].T,
        }
    in_maps = []
    for c in range(N_CORES):
        b, qs = c // 4, (c % 4) * QLEN
        in_maps.append({
            "xqT": np.ascontiguousarray(
                per_b[b]["qT"][:, qs:qs + QLEN]).astype(bf16),
            "xkT": per_b[b]["xkT"],
            "valp": per_b[b]["valp"],
            "maskT": np.ascontiguousarray(maskT[:, qs:qs + QLEN]),
            **common,
        })
    return in_maps


def get_module():
    if "nc" not in _CACHE:
        _CACHE["nc"] = _build_module()
    return _CACHE["nc"]


def kernel(key, query, value, mask, Wq, Wk, Wv, Wo, bo, **_):
    nc = get_module()
    in_maps = _prep_inputs(key, query, value, mask, Wq, Wk, Wv, Wo, bo)
    res = bass_utils.run_bass_kernel_spmd(
        nc, in_maps, core_ids=list(range(N_CORES)))
    full = np.empty((B, S, E), np.float32)
    for c in range(N_CORES):
        b, qs = c // 4, (c % 4) * QLEN
        full[b, qs:qs + QLEN, :] = res.results[c]["out"]
    return full

